# revision 16
# baseline (speedup 1.0000x reference)
"""GQA kernel for Trainium2, 8 NeuronCores.

Sharding: 2 batches x 4 head-shards. Each core handles one batch and
2 KV groups (= 8 Q heads, 512 of the 2048 head-concat columns).
Per core the out-projection produces a partial [S, D] sum; the host
adds the 4 partials per batch (the "all-reduce after out_proj") + bo.

All matmul operands are fp16 (full-rate PE; f32r runs at half clock).
All inputs are repacked on the host into SBUF-layout [128, N] panels
so each loads with a couple of large DMAs (a dma_start costs ~650ns
of serialized issue time, so many small DMAs throttle the prefix).
xT stays SBUF-resident (64 KB/partition) so projections never wait on
DMA. Phase B is a single software pipeline around the continuous
score->exp stream:

  prefix:  K/V projections (+ v transposes) for all of S, plus the
           q-projection for the first head pair.
  steady state, one "group" (2 key chunks x 512 queries) per step:
    - ctx matmuls for the group LAG steps behind (their exps are done)
    - 2 q-projection matmuls for the NEXT head pair
    - out-projection quarter-tiles of an already-finished query block
    - score matmuls (two heads as concurrent row-tiles) + 2 exps
  When a pair's last ctx group lands, its ctx PSUM is copied to SBUF
  staging at once (freeing the PSUM banks for the next pair); the
  softmax normalization runs off the staging copy with the reciprocal
  computed on ACT as exp(-ln(sums)) so the DVE stays light.

Device-side math per core (b = batch, columns c0 = shard*512):
  qT2[pr] = (x_b @ Wq[:, c0+128pr : +128] + bq).T        [128, S]  (head pair)
  kT2[g]  = ((x_b @ Wk[:, ...] + bk)).T, duplicated on both halves [128, S]
  v       = x_b @ Wv + bv, stored per key-chunk as [64 v_g | 1]  [128, 16*130]
  scT     = kT chunk^T x qT  (keys on partitions)               [128, 512]
  eT      = exp(scT / 8)   (no max subtraction: scores ~ N(0,1))
  ctxT    = [v_g | 1]^T @ eT -> rows 0..63 ctx^T, row 64 = softmax sums
  ctxT'   = ctxT * exp(-ln(sums))
  y_part  = sum_pr ctxT2'[pr]^T @ Wo[...]                        [S, D]
"""

import sys

sys.path.insert(0, "/opt/trn_rl_repo")

import numpy as np

N_CORES = 8
S = 2048  # sequence length
D = 2048  # d_model
HD = 64  # head dim
GL = 2  # local KV groups per core
CPS = 512  # q/out columns per shard
KPS = 128  # kv columns per shard
SCALE = 1.0 / 8.0  # 1/sqrt(HD)
DC = D // 128  # 16 contraction chunks for projections

_CACHE = {}


def _build_bass():
    import concourse.bass as bass
    import concourse.bacc as bacc
    import concourse.mybir as mybir
    import concourse.tile as tile
    from concourse.masks import make_identity

    f32 = mybir.dt.float32
    f32r = mybir.dt.float32r
    f16 = mybir.dt.float16
    ALU = mybir.AluOpType
    ACTF = mybir.ActivationFunctionType

    nc = bacc.Bacc("TRN2", target_bir_lowering=False)

    # host-repacked SBUF-layout panels (see make_in_maps)
    xTp = nc.dram_tensor("xTp", [128, DC * S], f16, kind="ExternalInput")
    Wqp = nc.dram_tensor("Wqp", [128, DC * CPS], f16, kind="ExternalInput")
    Wkvp = nc.dram_tensor("Wkvp", [128, DC * 256], f16, kind="ExternalInput")
    Wop = nc.dram_tensor("Wop", [128, 4 * D], f16, kind="ExternalInput")
    bq = nc.dram_tensor("bq", [CPS], f32, kind="ExternalInput")
    bk = nc.dram_tensor("bk", [KPS], f32, kind="ExternalInput")
    bv = nc.dram_tensor("bv", [KPS], f32, kind="ExternalInput")
    y = nc.dram_tensor("y", [S, D], f32, kind="ExternalOutput")

    SC = S // 128  # 16 key chunks
    QB = 4  # query blocks of 512 in attention
    QBS = S // QB
    KGRP = 2  # key chunks per exp group (psum tile = 2 banks f32)
    NGRP = SC // KGRP  # groups per (pair, qb)
    NPAIR = 16  # (qb, pr) pairs
    LAG = 6  # ctx trails scores by this many groups (global pipeline)

    with tile.TileContext(nc) as tc:
        with (
            tc.tile_pool(name="persist", bufs=1) as pp,
            tc.tile_pool(name="psQ", bufs=1, space=bass.MemorySpace.PSUM) as psQ,
        ):
            # ---- persistent SBUF tensors (per-partition KB) ----
            xTall = pp.tile([128, DC * S], f16, tag="xTall")  # 64
            wq_sb = pp.tile([128, DC * CPS], f16, tag="wq_sb")  # 16
            wo_sb = pp.tile([128, 4 * D], f16, tag="wo_sb")  # 16
            qT2 = [pp.tile([128, S], f16, name=f"qT{p}", tag=f"qT{p}") for p in range(4)]  # 16
            kT2 = [pp.tile([128, S], f16, name=f"kT{g}", tag=f"kT{g}") for g in range(GL)]  # 8
            v_sb = pp.tile([128, SC * 130], f16, tag="v_sb")  # 4.1
            ctxT2 = [pp.tile([128, S], f16, name=f"ctxT{p}", tag=f"ctxT{p}") for p in range(4)]  # 16
            bqs = [pp.tile([128, 1], f32, name=f"bq{t}", tag=f"bq{t}") for t in range(4)]
            bks = pp.tile([128, 1], f32, tag="bks")
            bvs = pp.tile([128, 1], f32, tag="bvs")
            ident = pp.tile([128, 128], f16, tag="ident")
            vones = pp.tile([128, 1], f16, tag="vones")
            ident_f32 = pp.tile([128, 128], f32, tag="ident_f32")

            def xts(dc):
                return xTall[:, dc * S : (dc + 1) * S]

            nc.gpsimd.memset(vones[:], 1.0)
            for k in range(2 * SC):
                nc.vector.tensor_copy(v_sb[:, 64 + 65 * k : 65 + 65 * k], vones[:])
            make_identity(nc, ident_f32[:])
            nc.vector.tensor_copy(ident[:], ident_f32[:])

            for t in range(4):
                nc.sync.dma_start(bqs[t][:], bq[t * 128 : (t + 1) * 128])
            nc.sync.dma_start(bks[:], bk[:])
            nc.sync.dma_start(bvs[:], bv[:])

            # q-projection accumulators, one live at a time
            qacc = {}

            def qproj_mms(pair, dcs):
                qb, pr = divmod(pair, 4)
                if pair not in qacc:
                    qacc[pair] = psQ.tile([128, 512], f32, name=f"qa{pair}", tag="qacc")
                for dc in dcs:
                    nc.tensor.matmul(
                        qacc[pair][:],
                        wq_sb[:, dc * CPS + pr * 128 : dc * CPS + (pr + 1) * 128],
                        xts(dc)[:, qb * 512 : (qb + 1) * 512],
                        start=(dc == 0),
                        stop=(dc == DC - 1),
                    )

            def qproj_finish(pair):
                qb, pr = divmod(pair, 4)
                nc.vector.tensor_scalar_add(
                    qT2[pr][:, qb * 512 : (qb + 1) * 512], qacc[pair][:], bqs[pr][:]
                )
                del qacc[pair]

            # ---- prefix: loads, K/V projections, qproj(pair 0) ----
            with (
                tc.tile_pool(name="wkv", bufs=1) as wkvp_p,
                tc.tile_pool(name="stA", bufs=4) as st,
                tc.tile_pool(name="psKV", bufs=1, space=bass.MemorySpace.PSUM) as psKV,
                tc.tile_pool(name="psT", bufs=2, space=bass.MemorySpace.PSUM) as psT,
            ):
                wkv_sb = wkvp_p.tile([128, DC * 256], f16, tag="wkv_sb")  # 8
                # big DMAs, first-needed first; x split in 4 to engage
                # multiple queues
                nc.sync.dma_start(wkv_sb[:], Wkvp[:, :])
                XQ = DC * S // 4
                nc.sync.dma_start(xTall[:, 0 * XQ : 1 * XQ], xTp[:, 0 * XQ : 1 * XQ])
                nc.sync.dma_start(wq_sb[:, 0 : DC * CPS // 2], Wqp[:, 0 : DC * CPS // 2])
                nc.sync.dma_start(xTall[:, 1 * XQ : 2 * XQ], xTp[:, 1 * XQ : 2 * XQ])
                nc.sync.dma_start(
                    wq_sb[:, DC * CPS // 2 : DC * CPS], Wqp[:, DC * CPS // 2 : DC * CPS]
                )
                nc.sync.dma_start(xTall[:, 2 * XQ : 3 * XQ], xTp[:, 2 * XQ : 3 * XQ])
                nc.sync.dma_start(wo_sb[:, 0 : 2 * D], Wop[:, 0 : 2 * D])
                nc.sync.dma_start(xTall[:, 3 * XQ : 4 * XQ], xTp[:, 3 * XQ : 4 * XQ])
                nc.sync.dma_start(wo_sb[:, 2 * D : 4 * D], Wop[:, 2 * D : 4 * D])

                for sq in range(4):
                    s0 = sq * 512
                    ssl = slice(s0, s0 + 512)
                    kps = psKV.tile([128, 512], f32, name=f"kp{sq}", tag="kp")
                    vps = psKV.tile([128, 512], f32, name=f"vp{sq}", tag="vp")
                    for dc in range(DC):
                        nc.tensor.matmul(
                            kps[:], wkv_sb[:, dc * 256 : dc * 256 + 128], xts(dc)[:, ssl],
                            start=(dc == 0), stop=(dc == DC - 1),
                        )
                        nc.tensor.matmul(
                            vps[:], wkv_sb[:, dc * 256 + 128 : dc * 256 + 256], xts(dc)[:, ssl],
                            start=(dc == 0), stop=(dc == DC - 1),
                        )
                        if sq == 0:
                            qproj_mms(0, [dc])
                    for g in range(GL):
                        gs = slice(g * 64, (g + 1) * 64)
                        for half in range(2):
                            hs = slice(half * 64, (half + 1) * 64)
                            nc.vector.tensor_scalar_add(
                                kT2[g][hs, ssl], kps[gs, :], bks[gs, :]
                            )
                    vt = st.tile([128, 512], f16, tag="vt")
                    nc.vector.tensor_scalar_add(vt[:], vps[:], bvs[:])
                    for c4 in range(4):
                        tck = sq * 4 + c4
                        tp = psT.tile([128, 128], f16, tag="vtp")
                        nc.tensor.transpose(tp[:], vt[:, c4 * 128 : (c4 + 1) * 128], ident[:])
                        for g in range(GL):
                            nc.vector.tensor_copy(
                                v_sb[:, tck * 130 + g * 65 : tck * 130 + g * 65 + 64],
                                tp[:, g * 64 : (g + 1) * 64],
                            )
                qproj_finish(0)

            # ---- phase B: pipelined attention + spread out-projection ----
            with (
                tc.tile_pool(name="psS", bufs=2, space=bass.MemorySpace.PSUM) as psS,
                tc.tile_pool(name="psC", bufs=1, space=bass.MemorySpace.PSUM) as psC,
                tc.tile_pool(name="psO", bufs=1, space=bass.MemorySpace.PSUM) as psO,
                tc.tile_pool(name="eT", bufs=LAG + 1) as ep,
                tc.tile_pool(name="stage", bufs=2) as sg,
                tc.tile_pool(name="rc", bufs=2) as rp,
                tc.tile_pool(name="stC", bufs=2) as so,
            ):
                ctx_tiles = {}  # pair -> [ctx psum tile per head]
                eT_store = {}  # global group G -> [eT tile per head]
                osb_tiles = {}  # qt -> output staging tile

                def scores(G):
                    pair, grp = divmod(G, NGRP)
                    qb, pr = divmod(pair, 4)
                    g = pr // 2
                    qsl = slice(qb * QBS, (qb + 1) * QBS)
                    sc2 = [psS.tile([128, KGRP * QBS], f32, name=f"sc{i}", tag="sc") for i in range(2)]
                    for j in range(KGRP):
                        kc = grp * KGRP + j
                        for h2 in range(2):
                            hs = slice(h2 * 64, (h2 + 1) * 64)
                            nc.tensor.matmul(
                                sc2[h2][:, j * QBS : (j + 1) * QBS],
                                kT2[g][hs, kc * 128 : (kc + 1) * 128],
                                qT2[pr][hs, qsl],
                                start=True,
                                stop=True,
                                tile_position=(h2 * 64, 0),
                            )
                    eT2 = [ep.tile([128, KGRP * QBS], f16, name=f"eT{h2}", tag=f"eT{h2}") for h2 in range(2)]
                    for h2 in range(2):
                        nc.scalar.activation(eT2[h2][:], sc2[h2][:], ACTF.Exp, scale=SCALE)
                    eT_store[G] = eT2

                def ctx(G):
                    pair, grp = divmod(G, NGRP)
                    g = (pair % 4) // 2
                    if pair not in ctx_tiles:
                        ctx_tiles[pair] = [
                            psC.tile([65, QBS], f32, name=f"ctx{h2}", tag=f"ctx{h2}")
                            for h2 in range(2)
                        ]
                    eT2 = eT_store.pop(G)
                    for j in range(KGRP):
                        kc = grp * KGRP + j
                        vsl = slice(kc * 130 + g * 65, kc * 130 + (g + 1) * 65)
                        for h2 in range(2):
                            nc.tensor.matmul(
                                ctx_tiles[pair][h2][:],
                                v_sb[:, vsl],
                                eT2[h2][:, j * QBS : (j + 1) * QBS],
                                start=(kc == 0),
                                stop=(kc == SC - 1),
                            )
                    if grp == NGRP - 1:
                        normalize(pair)

                def normalize(pair):
                    qb, pr = divmod(pair, 4)
                    qsl = slice(qb * QBS, (qb + 1) * QBS)
                    ctxp = ctx_tiles.pop(pair)
                    # copy to SBUF staging right away to release the PSUM
                    # banks for the next pair; normalize off the staging copy
                    stg = [sg.tile([65, QBS], f32, name=f"stg{h2}", tag=f"stg{h2}") for h2 in range(2)]
                    for h2 in range(2):
                        nc.vector.tensor_copy(stg[h2][:], ctxp[h2][:])
                    for h2 in range(2):
                        # 1/sums on ACT as exp(-ln(sums)); DVE reciprocal is
                        # 8 cycles/elem and would gate the whole chain
                        lns = rp.tile([1, QBS], f32, tag="lns")
                        nc.scalar.activation(lns[:], stg[h2][64:65, :], ACTF.Ln)
                        recip = rp.tile([1, QBS], f32r, tag="recip")
                        nc.scalar.activation(recip[:], lns[:], ACTF.Exp, scale=-1.0)
                        bc = rp.tile([64, QBS], f32r, tag="bc")
                        nc.gpsimd.partition_broadcast(bc[:], recip[:])
                        hs = slice(h2 * 64, (h2 + 1) * 64)
                        nc.vector.tensor_tensor(
                            out=ctxT2[pr][hs, qsl],
                            in0=stg[h2][0:64, :],
                            in1=bc[:],
                            op=ALU.mult,
                        )

                def outproj_quarter(qb, qi):
                    qt = qb * 4 + qi // 4
                    qtr = qi % 4
                    ops = psO.tile([128, 512], f32, tag="out")
                    for p in range(4):
                        nc.tensor.matmul(
                            ops[:],
                            ctxT2[p][:, qt * 128 : (qt + 1) * 128],
                            wo_sb[:, p * D + qtr * 512 : p * D + (qtr + 1) * 512],
                            start=(p == 0),
                            stop=(p == 3),
                        )
                    if qt not in osb_tiles:
                        osb_tiles[qt] = so.tile([128, D], f32, name=f"osb{qt}", tag="osb")
                    osb = osb_tiles[qt]
                    nc.vector.tensor_copy(osb[:, qtr * 512 : (qtr + 1) * 512], ops[:])
                    if qtr == 3:
                        nc.sync.dma_start(y[qt * 128 : (qt + 1) * 128, :], osb[:])
                        del osb_tiles[qt]

                NG = NPAIR * NGRP  # 128 global groups
                for G in range(NG):
                    pair, grp = divmod(G, NGRP)
                    if G - LAG >= 0:
                        ctx(G - LAG)
                    # q-projection for the next pair, 2 dc chunks per group
                    if pair + 1 < NPAIR:
                        qproj_mms(pair + 1, [2 * grp, 2 * grp + 1])
                        if grp == NGRP - 1:
                            qproj_finish(pair + 1)
                    # out-projection of the previous query block, 1 quarter
                    # per group, spread over the 3rd+4th pairs of this block
                    # (the previous block's last normalize chain is then done)
                    if pair % 4 in (2, 3) and pair >= 6:
                        qi = (pair % 4 - 2) * NGRP + grp
                        outproj_quarter(pair // 4 - 1, qi)
                    scores(G)
                # drain: remaining ctx groups, then last block's out-projection
                for G in range(NG - LAG, NG):
                    ctx(G)
                for qi in range(16):
                    outproj_quarter(3, qi)

    nc.compile()
    return nc


def _get_nc():
    if "nc" not in _CACHE:
        _CACHE["nc"] = _build_bass()
    return _CACHE["nc"]


def _pack(panel):
    """[DC*128, N] row-chunked weight/activation -> SBUF layout [128, DC*N]."""
    dc = panel.shape[0] // 128
    return np.ascontiguousarray(
        panel.reshape(dc, 128, panel.shape[1]).transpose(1, 0, 2).reshape(128, -1)
    )


def make_in_maps(x, Wq, bq, Wk, bk, Wv, bv, Wo):
    f16 = np.float16
    xTb = [_pack(np.ascontiguousarray(x[b].T).astype(f16)) for b in range(2)]
    Wq16 = Wq.astype(f16)
    Wk16 = Wk.astype(f16)
    Wv16 = Wv.astype(f16)
    Wo16 = Wo.astype(f16)
    in_maps = []
    for c in range(N_CORES):
        b, sh = divmod(c, 4)
        wk = Wk16[:, sh * KPS : (sh + 1) * KPS].reshape(DC, 128, KPS)
        wv = Wv16[:, sh * KPS : (sh + 1) * KPS].reshape(DC, 128, KPS)
        wkv = np.concatenate([wk, wv], axis=2)  # [DC, 128, 256]
        in_maps.append(
            {
                "xTp": xTb[b],
                "Wqp": _pack(Wq16[:, sh * CPS : (sh + 1) * CPS]),
                "Wkvp": np.ascontiguousarray(
                    wkv.transpose(1, 0, 2).reshape(128, DC * 256)
                ),
                "Wop": _pack(Wo16[sh * CPS : (sh + 1) * CPS, :]),
                "bq": np.ascontiguousarray(bq[sh * CPS : (sh + 1) * CPS]),
                "bk": np.ascontiguousarray(bk[sh * KPS : (sh + 1) * KPS]),
                "bv": np.ascontiguousarray(bv[sh * KPS : (sh + 1) * KPS]),
            }
        )
    return in_maps


def kernel(x, Wq, bq, Wk, bk, Wv, bv, Wo, bo):
    from concourse.bass_utils import run_bass_kernel_spmd

    x = np.asarray(x, dtype=np.float32)
    Wq = np.asarray(Wq, dtype=np.float32)
    Wk = np.asarray(Wk, dtype=np.float32)
    Wv = np.asarray(Wv, dtype=np.float32)
    Wo = np.asarray(Wo, dtype=np.float32)
    bq = np.asarray(bq, dtype=np.float32)
    bk = np.asarray(bk, dtype=np.float32)
    bv = np.asarray(bv, dtype=np.float32)
    bo = np.asarray(bo, dtype=np.float32)

    in_maps = make_in_maps(x, Wq, bq, Wk, bk, Wv, bv, Wo)
    nc = _get_nc()
    res = run_bass_kernel_spmd(nc, in_maps, core_ids=list(range(N_CORES)))
    out = np.zeros((2, S, D), dtype=np.float32)
    for c in range(N_CORES):
        b = c // 4
        out[b] += res.results[c]["y"]
    out += bo
    return out


# revision 31
# speedup vs baseline: 1.1238x; 1.1238x over previous
"""GQA kernel for Trainium2, 8 NeuronCores.

Sharding: 2 batches x 4 head-shards. Each core handles one batch and
2 KV groups (= 8 Q heads, 512 of the 2048 head-concat columns).
Per core the out-projection produces a partial [S, D] sum; the host
adds the 4 partials per batch (the "all-reduce after out_proj") + bo.

All matmul operands are fp16 (full-rate PE; f32r runs at half clock).
All inputs are repacked on the host into SBUF-layout [128, N] panels
so each loads with a couple of large DMAs (a dma_start costs ~650ns
of serialized issue time, so many small DMAs throttle the prefix).
xT stays SBUF-resident (64 KB/partition) so projections never wait on
DMA. Phase B is a single software pipeline around the continuous
score->exp stream:

  prefix:  K/V projections (+ v transposes) for all of S, plus the
           q-projection for the first head pair.
  steady state, one "group" (2 key chunks x 512 queries) per step:
    - ctx matmuls for the group LAG steps behind (their exps are done)
    - 2 q-projection matmuls for the NEXT head pair
    - out-projection quarter-tiles of an already-finished query block
    - score matmuls (two heads as concurrent row-tiles) + 2 exps
  When a pair's last ctx group lands, its ctx PSUM is copied to SBUF
  staging at once (freeing the PSUM banks for the next pair); the
  softmax normalization runs off the staging copy with the reciprocal
  computed on ACT as exp(-ln(sums)) so the DVE stays light.

Device-side math per core (b = batch, columns c0 = shard*512):
  qT2[pr] = (x_b @ Wq[:, c0+128pr : +128] + bq).T        [128, S]  (head pair)
  kT2[g]  = ((x_b @ Wk[:, ...] + bk)).T, duplicated on both halves [128, S]
  v       = x_b @ Wv + bv, stored per key-chunk as [64 v_g | 1]  [128, 16*130]
  scT     = kT chunk^T x qT  (keys on partitions)               [128, 512]
  eT      = exp(scT / 8)   (no max subtraction: scores ~ N(0,1))
  ctxT    = [v_g | 1]^T @ eT -> rows 0..63 ctx^T, row 64 = softmax sums
  ctxT'   = ctxT * exp(-ln(sums))
  y_part  = sum_pr ctxT2'[pr]^T @ Wo[...]                        [S, D]
"""

import sys

sys.path.insert(0, "/opt/trn_rl_repo")

import numpy as np

N_CORES = 8
S = 2048  # sequence length
D = 2048  # d_model
HD = 64  # head dim
GL = 2  # local KV groups per core
CPS = 512  # q/out columns per shard
KPS = 128  # kv columns per shard
SCALE = 1.0 / 8.0  # 1/sqrt(HD)
DC = D // 128  # 16 contraction chunks for projections

_CACHE = {}


def _build_bass():
    import concourse.bass as bass
    import concourse.bacc as bacc
    import concourse.mybir as mybir
    import concourse.tile as tile
    from concourse.masks import make_identity

    f32 = mybir.dt.float32
    f32r = mybir.dt.float32r
    f16 = mybir.dt.float16
    ALU = mybir.AluOpType
    ACTF = mybir.ActivationFunctionType

    nc = bacc.Bacc("TRN2", target_bir_lowering=False)

    # host-repacked SBUF-layout panels (see make_in_maps)
    xTp = nc.dram_tensor("xTp", [128, DC * S], f16, kind="ExternalInput")
    Wqp = nc.dram_tensor("Wqp", [128, DC * CPS], f16, kind="ExternalInput")
    Wkvp = nc.dram_tensor("Wkvp", [128, DC * 256], f16, kind="ExternalInput")
    Wop = nc.dram_tensor("Wop", [128, 4 * D], f16, kind="ExternalInput")
    bq = nc.dram_tensor("bq", [CPS], f32, kind="ExternalInput")
    bk = nc.dram_tensor("bk", [KPS], f32, kind="ExternalInput")
    bv = nc.dram_tensor("bv", [KPS], f32, kind="ExternalInput")
    y = nc.dram_tensor("y", [S, D], f32, kind="ExternalOutput")

    SC = S // 128  # 16 key chunks
    QB = 4  # query blocks of 512 in attention
    QBS = S // QB
    KGRP = 2  # key chunks per exp group (psum tile = 2 banks f32)
    NGRP = SC // KGRP  # groups per (pair, qb)
    NPAIR = 16  # (qb, pr) pairs
    LAG = 6  # ctx trails scores by this many groups (global pipeline)

    with tile.TileContext(nc) as tc:
        with (
            tc.tile_pool(name="persist", bufs=1) as pp,
            tc.tile_pool(name="psQ", bufs=1, space=bass.MemorySpace.PSUM) as psQ,
        ):
            # ---- persistent SBUF tensors (per-partition KB) ----
            xTall = pp.tile([128, DC * S], f16, tag="xTall")  # 64
            wq_sb = pp.tile([128, DC * CPS], f16, tag="wq_sb")  # 16
            wo_sb = pp.tile([128, 4 * D], f16, tag="wo_sb")  # 16
            qT2 = [pp.tile([128, S], f16, name=f"qT{p}", tag=f"qT{p}") for p in range(4)]  # 16
            kT2 = [pp.tile([128, S], f16, name=f"kT{g}", tag=f"kT{g}") for g in range(GL)]  # 8
            v_sb = pp.tile([128, SC * 130], f16, tag="v_sb")  # 4.1
            ctxT2 = [pp.tile([128, S], f16, name=f"ctxT{p}", tag=f"ctxT{p}") for p in range(4)]  # 16
            bqs = [pp.tile([128, 1], f32, name=f"bq{t}", tag=f"bq{t}") for t in range(4)]
            bks = pp.tile([128, 1], f32, tag="bks")
            bvs = pp.tile([128, 1], f32, tag="bvs")
            ident = pp.tile([128, 128], f16, tag="ident")
            vones = pp.tile([128, 1], f16, tag="vones")
            ident_f32 = pp.tile([128, 128], f32, tag="ident_f32")

            def xs(sq, dc):
                c0 = (sq * DC + dc) * 512
                return xTall[:, c0 : c0 + 512]

            nc.gpsimd.memset(vones[:], 1.0)
            for k in range(2 * SC):
                nc.vector.tensor_copy(v_sb[:, 64 + 65 * k : 65 + 65 * k], vones[:])
            make_identity(nc, ident_f32[:])
            nc.vector.tensor_copy(ident[:], ident_f32[:])

            for t in range(4):
                nc.sync.dma_start(bqs[t][:], bq[t * 128 : (t + 1) * 128])
            nc.sync.dma_start(bks[:], bk[:])
            nc.sync.dma_start(bvs[:], bv[:])

            # q-projection accumulators, one live at a time
            qacc = {}

            def qproj_mms(pair, dcs):
                qb, pr = divmod(pair, 4)
                if pair not in qacc:
                    qacc[pair] = psQ.tile([128, 512], f32, name=f"qa{pair}", tag="qacc")
                for dc in dcs:
                    nc.tensor.matmul(
                        qacc[pair][:],
                        wq_sb[:, dc * CPS + pr * 128 : dc * CPS + (pr + 1) * 128],
                        xs(qb, dc)[:],
                        start=(dc == 0),
                        stop=(dc == DC - 1),
                    )

            def qproj_finish(pair):
                # on ACT (Copy + bias): a DVE op here would queue behind the
                # previous pair's reciprocals and stall the next q-projection
                qb, pr = divmod(pair, 4)
                nc.scalar.activation(
                    qT2[pr][:, qb * 512 : (qb + 1) * 512],
                    qacc[pair][:],
                    ACTF.Identity,
                    bias=bqs[pr][:],
                )
                del qacc[pair]

            # ---- prefix: loads, K/V projections, qproj(pair 0) ----
            with (
                tc.tile_pool(name="wkv", bufs=1) as wkvp_p,
                tc.tile_pool(name="stA", bufs=4) as st,
                tc.tile_pool(name="psKV", bufs=1, space=bass.MemorySpace.PSUM) as psKV,
                tc.tile_pool(name="psT", bufs=2, space=bass.MemorySpace.PSUM) as psT,
            ):
                wkv_sb = wkvp_p.tile([128, DC * 256], f16, tag="wkv_sb")  # 8
                # big DMAs, first-needed first; x split in 4 to engage
                # multiple queues
                # x is packed sq-major on the host, so the K/V pass for
                # sequence block sq only waits on its own quarter DMA
                nc.sync.dma_start(wkv_sb[:], Wkvp[:, :])
                XQ = DC * S // 4
                nc.sync.dma_start(xTall[:, 0 * XQ : 1 * XQ], xTp[:, 0 * XQ : 1 * XQ])
                nc.sync.dma_start(wq_sb[:, 0 : DC * CPS // 2], Wqp[:, 0 : DC * CPS // 2])
                nc.sync.dma_start(xTall[:, 1 * XQ : 2 * XQ], xTp[:, 1 * XQ : 2 * XQ])
                nc.sync.dma_start(
                    wq_sb[:, DC * CPS // 2 : DC * CPS], Wqp[:, DC * CPS // 2 : DC * CPS]
                )
                nc.sync.dma_start(xTall[:, 2 * XQ : 3 * XQ], xTp[:, 2 * XQ : 3 * XQ])
                nc.sync.dma_start(wo_sb[:, 0 : 2 * D], Wop[:, 0 : 2 * D])
                nc.sync.dma_start(xTall[:, 3 * XQ : 4 * XQ], xTp[:, 3 * XQ : 4 * XQ])
                nc.sync.dma_start(wo_sb[:, 2 * D : 4 * D], Wop[:, 2 * D : 4 * D])

                for sq in range(4):
                    s0 = sq * 512
                    ssl = slice(s0, s0 + 512)
                    kps = psKV.tile([128, 512], f32, name=f"kp{sq}", tag="kp")
                    vps = psKV.tile([128, 512], f32, name=f"vp{sq}", tag="vp")
                    for dc in range(DC):
                        nc.tensor.matmul(
                            kps[:], wkv_sb[:, dc * 256 : dc * 256 + 128], xs(sq, dc)[:],
                            start=(dc == 0), stop=(dc == DC - 1),
                        )
                        nc.tensor.matmul(
                            vps[:], wkv_sb[:, dc * 256 + 128 : dc * 256 + 256], xs(sq, dc)[:],
                            start=(dc == 0), stop=(dc == DC - 1),
                        )
                        if sq == 0:
                            qproj_mms(0, [dc])
                    for g in range(GL):
                        gs = slice(g * 64, (g + 1) * 64)
                        for half in range(2):
                            hs = slice(half * 64, (half + 1) * 64)
                            nc.vector.tensor_scalar_add(
                                kT2[g][hs, ssl], kps[gs, :], bks[gs, :]
                            )
                    vt = st.tile([128, 512], f16, tag="vt")
                    nc.vector.tensor_scalar_add(vt[:], vps[:], bvs[:])
                    for c4 in range(4):
                        tck = sq * 4 + c4
                        tp = psT.tile([128, 128], f16, tag="vtp")
                        nc.tensor.transpose(tp[:], vt[:, c4 * 128 : (c4 + 1) * 128], ident[:])
                        for g in range(GL):
                            nc.vector.tensor_copy(
                                v_sb[:, tck * 130 + g * 65 : tck * 130 + g * 65 + 64],
                                tp[:, g * 64 : (g + 1) * 64],
                            )
                qproj_finish(0)

            # ---- phase B: pipelined attention + spread out-projection ----
            with (
                tc.tile_pool(name="psS", bufs=2, space=bass.MemorySpace.PSUM) as psS,
                tc.tile_pool(name="psC", bufs=1, space=bass.MemorySpace.PSUM) as psC,
                tc.tile_pool(name="psO", bufs=1, space=bass.MemorySpace.PSUM) as psO,
                tc.tile_pool(name="eT", bufs=LAG + 1) as ep,
                tc.tile_pool(name="stage", bufs=2) as sg,
                tc.tile_pool(name="rc", bufs=2) as rp,
                tc.tile_pool(name="stC", bufs=2) as so,
            ):
                ctx_tiles = {}  # pair -> [ctx psum tile per head]
                eT_store = {}  # global group G -> [eT tile per head]
                osb_tiles = {}  # qt -> output staging tile

                def scores(G):
                    pair, grp = divmod(G, NGRP)
                    qb, pr = divmod(pair, 4)
                    g = pr // 2
                    qsl = slice(qb * QBS, (qb + 1) * QBS)
                    sc2 = [psS.tile([128, KGRP * QBS], f32, name=f"sc{i}", tag="sc") for i in range(2)]
                    for j in range(KGRP):
                        kc = grp * KGRP + j
                        for h2 in range(2):
                            hs = slice(h2 * 64, (h2 + 1) * 64)
                            nc.tensor.matmul(
                                sc2[h2][:, j * QBS : (j + 1) * QBS],
                                kT2[g][hs, kc * 128 : (kc + 1) * 128],
                                qT2[pr][hs, qsl],
                                start=True,
                                stop=True,
                                tile_position=(h2 * 64, 0),
                            )
                    eT2 = [ep.tile([128, KGRP * QBS], f16, name=f"eT{h2}", tag=f"eT{h2}") for h2 in range(2)]
                    for h2 in range(2):
                        nc.scalar.activation(eT2[h2][:], sc2[h2][:], ACTF.Exp, scale=SCALE)
                    eT_store[G] = eT2

                def ctx(G):
                    pair, grp = divmod(G, NGRP)
                    g = (pair % 4) // 2
                    if pair not in ctx_tiles:
                        ctx_tiles[pair] = [
                            psC.tile([65, QBS], f32, name=f"ctx{h2}", tag=f"ctx{h2}")
                            for h2 in range(2)
                        ]
                    eT2 = eT_store.pop(G)
                    for j in range(KGRP):
                        kc = grp * KGRP + j
                        vsl = slice(kc * 130 + g * 65, kc * 130 + (g + 1) * 65)
                        for h2 in range(2):
                            nc.tensor.matmul(
                                ctx_tiles[pair][h2][:],
                                v_sb[:, vsl],
                                eT2[h2][:, j * QBS : (j + 1) * QBS],
                                start=(kc == 0),
                                stop=(kc == SC - 1),
                            )
                    if grp == NGRP - 1:
                        normalize(pair)

                def normalize(pair):
                    qb, pr = divmod(pair, 4)
                    qsl = slice(qb * QBS, (qb + 1) * QBS)
                    ctxp = ctx_tiles.pop(pair)
                    # copy to SBUF staging right away to release the PSUM
                    # banks for the next pair; normalize off the staging copy
                    stg = [sg.tile([65, QBS], f32, name=f"stg{h2}", tag=f"stg{h2}") for h2 in range(2)]
                    for h2 in range(2):
                        nc.vector.tensor_copy(stg[h2][:], ctxp[h2][:])
                    for h2 in range(2):
                        recip = rp.tile([1, QBS], f32r, tag="recip")
                        with nc.allow_low_precision(reason="f32r is 4-byte"):
                            nc.vector.reciprocal(recip[:], stg[h2][64:65, :])
                        bc = rp.tile([64, QBS], f32r, tag="bc")
                        nc.gpsimd.partition_broadcast(bc[:], recip[:])
                        hs = slice(h2 * 64, (h2 + 1) * 64)
                        nc.vector.tensor_tensor(
                            out=ctxT2[pr][hs, qsl],
                            in0=stg[h2][0:64, :],
                            in1=bc[:],
                            op=ALU.mult,
                        )

                def outproj_quarter(qb, qi):
                    qt = qb * 4 + qi // 4
                    qtr = qi % 4
                    ops = psO.tile([128, 512], f32, tag="out")
                    for p in range(4):
                        nc.tensor.matmul(
                            ops[:],
                            ctxT2[p][:, qt * 128 : (qt + 1) * 128],
                            wo_sb[:, p * D + qtr * 512 : p * D + (qtr + 1) * 512],
                            start=(p == 0),
                            stop=(p == 3),
                        )
                    if qt not in osb_tiles:
                        osb_tiles[qt] = so.tile([128, D], f32, name=f"osb{qt}", tag="osb")
                    osb = osb_tiles[qt]
                    nc.vector.tensor_copy(osb[:, qtr * 512 : (qtr + 1) * 512], ops[:])
                    if qtr == 3:
                        nc.sync.dma_start(y[qt * 128 : (qt + 1) * 128, :], osb[:])
                        del osb_tiles[qt]

                NG = NPAIR * NGRP  # 128 global groups
                for G in range(NG):
                    pair, grp = divmod(G, NGRP)
                    if G - LAG >= 0:
                        ctx(G - LAG)
                    # q-projection for the next pair, 2 dc chunks per group
                    if pair + 1 < NPAIR:
                        qproj_mms(pair + 1, [2 * grp, 2 * grp + 1])
                        if grp == NGRP - 1:
                            qproj_finish(pair + 1)
                    # out-projection of the previous query block, 1 quarter
                    # per group, spread over the 3rd+4th pairs of this block
                    # (the previous block's last normalize chain is then done)
                    if pair % 4 in (2, 3) and pair >= 6:
                        qi = (pair % 4 - 2) * NGRP + grp
                        outproj_quarter(pair // 4 - 1, qi)
                    scores(G)
                # drain: remaining ctx groups, then last block's out-projection
                for G in range(NG - LAG, NG):
                    ctx(G)
                for qi in range(16):
                    outproj_quarter(3, qi)

    nc.compile()
    return nc


def _get_nc():
    if "nc" not in _CACHE:
        _CACHE["nc"] = _build_bass()
    return _CACHE["nc"]


def _pack(panel):
    """[DC*128, N] row-chunked weight/activation -> SBUF layout [128, DC*N]."""
    dc = panel.shape[0] // 128
    return np.ascontiguousarray(
        panel.reshape(dc, 128, panel.shape[1]).transpose(1, 0, 2).reshape(128, -1)
    )


def _pack_x(xT):
    """[D, S] -> [128, (sq dc si)] with sq-major blocks of 512."""
    return np.ascontiguousarray(
        xT.reshape(DC, 128, 4, 512).transpose(1, 2, 0, 3).reshape(128, -1)
    )


def make_in_maps(x, Wq, bq, Wk, bk, Wv, bv, Wo):
    f16 = np.float16
    xTb = [_pack_x(np.ascontiguousarray(x[b].T).astype(f16)) for b in range(2)]
    Wq16 = Wq.astype(f16)
    Wk16 = Wk.astype(f16)
    Wv16 = Wv.astype(f16)
    Wo16 = Wo.astype(f16)
    in_maps = []
    for c in range(N_CORES):
        b, sh = divmod(c, 4)
        wk = Wk16[:, sh * KPS : (sh + 1) * KPS].reshape(DC, 128, KPS)
        wv = Wv16[:, sh * KPS : (sh + 1) * KPS].reshape(DC, 128, KPS)
        wkv = np.concatenate([wk, wv], axis=2)  # [DC, 128, 256]
        in_maps.append(
            {
                "xTp": xTb[b],
                "Wqp": _pack(Wq16[:, sh * CPS : (sh + 1) * CPS]),
                "Wkvp": np.ascontiguousarray(
                    wkv.transpose(1, 0, 2).reshape(128, DC * 256)
                ),
                "Wop": _pack(Wo16[sh * CPS : (sh + 1) * CPS, :]),
                "bq": np.ascontiguousarray(bq[sh * CPS : (sh + 1) * CPS]),
                "bk": np.ascontiguousarray(bk[sh * KPS : (sh + 1) * KPS]),
                "bv": np.ascontiguousarray(bv[sh * KPS : (sh + 1) * KPS]),
            }
        )
    return in_maps


def kernel(x, Wq, bq, Wk, bk, Wv, bv, Wo, bo):
    from concourse.bass_utils import run_bass_kernel_spmd

    x = np.asarray(x, dtype=np.float32)
    Wq = np.asarray(Wq, dtype=np.float32)
    Wk = np.asarray(Wk, dtype=np.float32)
    Wv = np.asarray(Wv, dtype=np.float32)
    Wo = np.asarray(Wo, dtype=np.float32)
    bq = np.asarray(bq, dtype=np.float32)
    bk = np.asarray(bk, dtype=np.float32)
    bv = np.asarray(bv, dtype=np.float32)
    bo = np.asarray(bo, dtype=np.float32)

    in_maps = make_in_maps(x, Wq, bq, Wk, bk, Wv, bv, Wo)
    nc = _get_nc()
    res = run_bass_kernel_spmd(nc, in_maps, core_ids=list(range(N_CORES)))
    out = np.zeros((2, S, D), dtype=np.float32)
    for c in range(N_CORES):
        b = c // 4
        out[b] += res.results[c]["y"]
    out += bo
    return out
